# revision 25
# baseline (speedup 1.0000x reference)
"""Multi-head attention (B=4, S=2048, D=1024, H=16, Dh=64) on 8 TRN2 cores.

Sharding: data-parallel over batch (4) x tensor-parallel over heads (2 groups
of 8). Core c handles batch c//2, head-group c%2 (heads g*8..g*8+7 via the
column split of W_q/W_k/W_v and row split of W_o). Each core emits a partial
output projection o^T [1024, 2048] bf16; the host sums the two head-group
partials per batch in f32 and transposes back.

All matmuls bf16 (inputs pre-cast on host), fp32 PSUM accumulation:
  Phase A (per head-pair ot): K^T, Q^T, V slices via bf16 matmuls. Chains run
  as PAIRS across psAO's two buffers, it-major interleaved, so each
  LDWEIGHTS hides under the partner chain's matmul stream.
  Phase B (per q-chunk qc, head-pair j):
    S^T tiles = K^T.T Q^T   (row-tiled head pairs at (0,0)/(64,0), K=64)
    P = exp(S/8)            (ACT per-ktile, PSUM[128,2,512] -> bf16)
    ctx^T += V.T P^T        (col-tiled pairs at (0,0)/(0,64), M=64)
    sums: DVE bf16 quarter-tree + col-tiled ones matmuls into one PSUM bank
    ctx^T *= 1/sums         (reciprocal_approx_fast + fused PSUM mul)
  Phase C: o^T = Wo_g^T.T @ ctx^T -> bf16 -> DMA out.

PSUM budget (8 banks): psS 2x2 + psC 1 + psR 1 + psAO 2 (projections and
output projection share a pool so phases overlap without false deps).
"""

import numpy as np

import concourse.bacc as bacc
import concourse.mybir as mybir
import concourse.tile as tile

D = 1024  # model dim
S = 2048  # sequence length
O = 512  # per-core projected dim (8 heads x 64)
IT = D // 128  # 8 input-dim tiles
NP = 4  # head pairs per core
QC = S // 512  # 4 q-chunks
KT = S // 128  # 16 k-tiles
VT = S // 128  # 16 s-tiles of V
MT = D // 128  # 8 output m-tiles
SCALE = 0.125  # 1/sqrt(64)

F32 = mybir.dt.float32
BF16 = mybir.dt.bfloat16


def build_kernel():
    nc = bacc.Bacc("TRN2", target_bir_lowering=False, debug=False, num_devices=8)
    xqt = nc.declare_dram_parameter("xqt", [QC, 128, IT, 512], BF16, isOutput=False)
    xkvt = nc.declare_dram_parameter("xkvt", [QC, 128, IT, 512], BF16, isOutput=False)
    wqt = nc.declare_dram_parameter("wqt", [128, IT, O], BF16, isOutput=False)
    wkt = nc.declare_dram_parameter("wkt", [128, IT, O], BF16, isOutput=False)
    wvt = nc.declare_dram_parameter("wvt", [128, IT, O], BF16, isOutput=False)
    wot = nc.declare_dram_parameter("wot", [128, O // 128, D], BF16, isOutput=False)
    ones = nc.declare_dram_parameter("ones", [128, 512], BF16, isOutput=False)
    otp = nc.declare_dram_parameter("otp", [D, S], BF16, isOutput=True)

    with tile.TileContext(nc) as tc:
        with (
            tc.tile_pool(name="persist", bufs=1) as persist,
            tc.tile_pool(name="ctxp", bufs=1) as ctxp,
            tc.tile_pool(name="pp", bufs=3) as ppp,
            tc.tile_pool(name="sump", bufs=2) as sump,
            tc.tile_pool(name="rp", bufs=2) as rp,
            tc.tile_pool(name="ots", bufs=3) as ots,
            tc.tile_pool(name="psS", bufs=2, space="PSUM") as psS,
            tc.tile_pool(name="psC", bufs=1, space="PSUM") as psC,
            tc.tile_pool(name="psR", bufs=1, space="PSUM") as psR,
            tc.tile_pool(name="psAO", bufs=2, space="PSUM") as psAO,
        ):
            qt_chunks = [
                [persist.tile([128, 512], BF16, name=f"qtc{j}_{c}") for c in range(QC)]
                for j in range(NP)
            ]
            kt_chunks = [
                [persist.tile([128, 512], BF16, name=f"ktc{j}_{c}") for c in range(QC)]
                for j in range(NP)
            ]
            v_tiles = [persist.tile([128, O], BF16, name=f"vt{t}") for t in range(VT)]
            wo_sb = persist.tile([128, O // 128, D], BF16)
            ones_full = persist.tile([128, 512], BF16)
            nc.sync.dma_start(out=ones_full, in_=ones[:, :])
            ones_bf = ones_full[:, 0:64]

            # PE warm-up filler: lowest-priority dummy matmuls the scheduler
            # slots into PE-idle gaps (DMA head) to keep the HAM clock warm.
            warm_ps = psR.tile([128, 512], F32, tag="psR", name="warm_ps")
            with tc.high_priority(offset=-(10**6)):
                for _ in range(16):
                    nc.tensor.matmul(
                        warm_ps[0:64, :],
                        ones_bf,
                        ones_full,
                        start=True,
                        stop=True,
                    )

            # ---- Phase A: projections (overlaps phase B via disjoint pools) ----
            with tc.tile_pool(name="wqx", bufs=1) as wqp:
                wq_sb = wqp.tile([128, IT, O], BF16)
                wk_sb = wqp.tile([128, IT, O], BF16)
                wv_sb = wqp.tile([128, IT, O], BF16)
                # All input DMAs on the sync HW-DGE queue, in the order the
                # projection pipeline consumes them (K first, then Q, V, O).
                # First tiles split in halves so the first proj chain starts
                # before its full tile lands.
                xq_c, xkv_c = [], []
                # wk + wq ride the scalar engine's HW-DGE ring (idle until
                # the first exp), halving the 4MB critical head with the
                # activations streaming on the sync ring concurrently.
                nc.scalar.dma_start(out=wk_sb, in_=wkt[:, :, :])
                nc.scalar.dma_start(out=wq_sb, in_=wqt[:, :, :])
                xkv_1 = wqp.tile([128, IT, 512], BF16, name="xkv0")
                for h in range(4):
                    sl = slice(2 * h, 2 * h + 2)
                    nc.sync.dma_start(out=xkv_1[:, sl, :], in_=xkvt[0, :, sl, :])
                xkv_c.append(xkv_1)
                xq_1 = wqp.tile([128, IT, 512], BF16, name="xq0")
                nc.sync.dma_start(out=xq_1, in_=xqt[0, :, :, :])
                xq_c.append(xq_1)
                for c in range(1, QC):
                    xkv_1 = wqp.tile([128, IT, 512], BF16, name=f"xkv{c}")
                    nc.sync.dma_start(out=xkv_1, in_=xkvt[c, :, :, :])
                    xkv_c.append(xkv_1)
                nc.sync.dma_start(out=wv_sb, in_=wvt[:, :, :])
                for c in range(1, QC):
                    xq_1 = wqp.tile([128, IT, 512], BF16, name=f"xq{c}")
                    nc.sync.dma_start(out=xq_1, in_=xqt[c, :, :, :])
                    xq_c.append(xq_1)
                nc.sync.dma_start(out=wo_sb, in_=wot[:, :, :])

                # ---- projection chain pairs: two psum chains (psAO bufs=2),
                # it-major interleaved so each LDW hides under the partner
                # chain's matmul stream. specs: (dst, 'kq'|'v', w/x args) ----
                _pn = [0]

                def proj_pair(specs):
                    pss = []
                    for _ in specs:
                        _pn[0] += 1
                        pss.append(
                            psAO.tile(
                                [128, 512], F32, tag="psAO", name=f"pj{_pn[0]}"
                            )
                        )
                    for it in range(IT):
                        for ps, spec in zip(pss, specs):
                            if spec[1] == "kq":
                                dst, _, w_sb, x_tile, ot = spec
                                nc.tensor.matmul(
                                    ps,
                                    w_sb[:, it, ot * 128 : (ot + 1) * 128],
                                    x_tile[:, it, :],
                                    start=(it == 0),
                                    stop=(it == IT - 1),
                                )
                            else:
                                dst, _, vt = spec
                                nc.tensor.matmul(
                                    ps,
                                    xkv_c[vt // 4][
                                        :, it, (vt % 4) * 128 : (vt % 4 + 1) * 128
                                    ],
                                    wv_sb[:, it, :],
                                    start=(it == 0),
                                    stop=(it == IT - 1),
                                )
                    for ps, spec in zip(pss, specs):
                        nc.vector.tensor_copy(spec[0], ps)

                def K(j, c):
                    return (kt_chunks[j][c], "kq", wk_sb, xkv_c[c], j)

                def Q(j, c):
                    return (qt_chunks[j][c], "kq", wq_sb, xq_c[c], j)

                def V(vt):
                    return (v_tiles[vt], "v", vt)

                # Head-pair 0's K/Q first so attention (and ACT) starts early;
                # V next (PV consumes v_tiles in kt order); rest of K/Q after.
                # Q chunks 2,3 are deferred into the attention phase (emitted
                # right before unit (2,0)) to offload the oversubscribed head.
                proj_pair([K(0, 0)])
                proj_pair([Q(0, 0)])
                proj_pair([K(0, 1), K(0, 2)])
                proj_pair([K(0, 3), Q(0, 1)])
                for vt in range(0, VT // 2, 2):
                    proj_pair([V(vt), V(vt + 1)])
                proj_pair([K(1, 0), K(1, 1)])
                proj_pair([K(1, 2), K(1, 3)])
                proj_pair([Q(1, 0), Q(1, 1)])
                for vt in range(VT // 2, VT, 2):
                    proj_pair([V(vt), V(vt + 1)])
                # K/Q for head-pairs 2,3 are interleaved into the first
                # attention units below: their consumers (units (qc,2),(qc,3))
                # run 50us+ later, safely past the LDWEIGHTS pull-ahead
                # window, and this unloads the early-phase PE crunch.

                # ---- Phase B: attention (pair-outer); Phase C: output proj ----
                ctx_tiles = [
                    ctxp.tile([128, NP, 512], BF16, name=f"ctx{c}") for c in range(QC)
                ]
                # First qc pair interleaved j-major (buys the projection
                # pipeline runway); second half qc-major. out_proj(qc) is
                # emitted as soon as ctx(qc) is complete so its chains fill
                # the ACT-bound attention phase's PE slack, not the tail.
                def out_proj(qc, tail=False):
                    qsl = slice(qc * 512, (qc + 1) * 512)
                    for mt in range(MT):
                        ps_o = psAO.tile(
                            [128, 512], F32, tag="psAO", name=f"ps_o{qc}_{mt}"
                        )
                        for jt in range(NP):
                            nc.tensor.matmul(
                                ps_o,
                                wo_sb[:, jt, mt * 128 : (mt + 1) * 128],
                                ctx_tiles[qc][:, jt, :],
                                start=(jt == 0),
                                stop=(jt == NP - 1),
                            )
                        ot_sb = ots.tile([128, 512], BF16, tag="ot")
                        if tail and mt % 2 == 1:
                            # Tail copies alternate onto the (now idle)
                            # scalar engine so psAO slots free twice as fast.
                            nc.scalar.activation(
                                out=ot_sb,
                                in_=ps_o,
                                func=mybir.ActivationFunctionType.Copy,
                            )
                        else:
                            nc.vector.tensor_copy(ot_sb, ps_o)
                        nc.sync.dma_start(
                            out=otp[mt * 128 : (mt + 1) * 128, qsl], in_=ot_sb
                        )

                unit_order = [(qc, j) for j in range(NP) for qc in (0, 1)]
                unit_order += [(qc, j) for qc in (2, 3) for j in range(NP)]
                for qc, j in unit_order:
                    if (qc, j) == (1, 0):
                        proj_pair([K(2, 0), K(2, 1)])
                        proj_pair([K(2, 2), K(2, 3)])
                    if (qc, j) == (0, 1):
                        proj_pair([Q(2, 0), Q(2, 1)])
                        proj_pair([K(3, 0), K(3, 1)])
                    if (qc, j) == (1, 1):
                        proj_pair([K(3, 2), K(3, 3)])
                        proj_pair([Q(3, 0), Q(3, 1)])
                    if (qc, j) == (2, 0):
                        for ot in range(NP):
                            proj_pair([Q(ot, 2), Q(ot, 3)])
                    if (qc, j) == (2, 1):
                        out_proj(0)
                    if (qc, j) == (2, 3):
                        out_proj(1)
                    if (qc, j) == (3, 1):
                        out_proj(2)
                    if True:
                        psum_ctx = psC.tile([128, 512], F32, tag="psC")
                        psum_r = psR.tile([128, 512], F32, tag="psR")
                        s1_tiles = []
                        sh_tiles = []
                        for qi in range(4):  # quarters of the k range
                            pq = ppp.tile([128, 4, 2, 512], BF16, tag="pp")
                            for ki in range(4):  # per-ktile ACT batches
                                kt = qi * 4 + ki
                                ps_s = psS.tile([128, 2, 512], F32, tag="psS")
                                with tc.high_priority(offset=50000):
                                    nc.tensor.matmul(
                                        ps_s[:, 0, :],
                                        kt_chunks[j][kt // 4][0:64, (kt % 4) * 128 : (kt % 4 + 1) * 128],
                                        qt_chunks[j][qc][0:64, :],
                                        start=True,
                                        stop=True,
                                        tile_position=(0, 0),
                                    )
                                    nc.tensor.matmul(
                                        ps_s[:, 1, :],
                                        kt_chunks[j][kt // 4][64:128, (kt % 4) * 128 : (kt % 4 + 1) * 128],
                                        qt_chunks[j][qc][64:128, :],
                                        start=True,
                                        stop=True,
                                        tile_position=(64, 0),
                                    )
                                    nc.scalar.activation(
                                        out=pq[:, ki, :, :],
                                        in_=ps_s[:, :, :],
                                        func=mybir.ActivationFunctionType.Exp,
                                        scale=SCALE,
                                    )
                            # PV + sums for this quarter
                            with tc.high_priority(offset=50000):
                                for ki in range(4):
                                    kt = qi * 4 + ki
                                    first = kt == 0
                                    last = kt == KT - 1
                                    nc.tensor.matmul(
                                        psum_ctx[0:64, :],
                                        v_tiles[kt][:, j * 128 : j * 128 + 64],
                                        pq[:, ki, 0, :],
                                        start=first,
                                        stop=last,
                                        tile_position=(0, 0),
                                    )
                                    nc.tensor.matmul(
                                        psum_ctx[64:128, :],
                                        v_tiles[kt][:, j * 128 + 64 : (j + 1) * 128],
                                        pq[:, ki, 1, :],
                                        start=first,
                                        stop=last,
                                        tile_position=(0, 64),
                                    )
                                tq = sump.tile([128, 2, 2, 512], BF16, tag="tq", bufs=1)
                                s1 = sump.tile([128, 2, 512], BF16, tag="s1")
                                with nc.allow_low_precision(reason="softmax sum partials"):
                                    nc.vector.tensor_add(
                                        tq, pq[:, 0:2, :, :], pq[:, 2:4, :, :]
                                    )
                                    nc.vector.tensor_add(
                                        s1, tq[:, 0, :, :], tq[:, 1, :, :]
                                    )
                                s1_tiles.append(s1)
                                if qi % 2 == 1:
                                    sh = sump.tile([128, 2, 512], BF16, tag="sh")
                                    with nc.allow_low_precision(reason="softmax sum partials"):
                                        nc.vector.tensor_add(
                                            sh, s1_tiles[qi - 1], s1_tiles[qi]
                                        )
                                    sh_tiles.append(sh)
                        s_all = sh_tiles[0]
                        with nc.allow_low_precision(reason="softmax sum partials"):
                            nc.vector.tensor_add(s_all, sh_tiles[0], sh_tiles[1])
                        with tc.high_priority(offset=50000):
                            nc.tensor.matmul(
                                psum_r[0:64, :],
                                ones_bf,
                                s_all[:, 0, :],
                                start=True,
                                stop=True,
                                tile_position=(0, 0),
                            )
                            nc.tensor.matmul(
                                psum_r[64:128, :],
                                ones_bf,
                                s_all[:, 1, :],
                                start=True,
                                stop=True,
                                tile_position=(0, 64),
                            )
                        # normalize
                        with tc.high_priority(offset=50000):
                            r_tile = rp.tile([128, 512], F32, tag="r")
                            nc.vector.reciprocal_approx_fast(out=r_tile, in_=psum_r)
                            with nc.allow_low_precision(reason="bf16 ctx for PE"):
                                nc.vector.tensor_mul(
                                    ctx_tiles[qc][:, j, :], psum_ctx, r_tile
                                )
                # Phase C tail: last q-chunk's output projection
                out_proj(3, tail=True)
    nc.compile()
    return nc


def make_in_maps(query_input, kv_input, W_q, W_k, W_v, W_o):
    import ml_dtypes

    bf16 = ml_dtypes.bfloat16
    q = np.asarray(query_input, dtype=np.float32).astype(bf16)
    kv = np.asarray(kv_input, dtype=np.float32).astype(bf16)
    W_q = np.asarray(W_q, dtype=np.float32).astype(bf16)
    W_k = np.asarray(W_k, dtype=np.float32).astype(bf16)
    W_v = np.asarray(W_v, dtype=np.float32).astype(bf16)
    W_o = np.asarray(W_o, dtype=np.float32).astype(bf16)
    ones = np.ones((128, 512), dtype=bf16)

    def tile_x(xt):  # [D, S] -> [QC, 128, IT, 512]
        return np.ascontiguousarray(
            xt.reshape(IT, 128, QC, 512).transpose(2, 1, 0, 3)
        )

    def tile_w(wt):  # [D, O] -> [128, IT, O]
        return np.ascontiguousarray(wt.reshape(IT, 128, O).transpose(1, 0, 2))

    in_maps = []
    for c in range(8):
        b, g = c // 2, c % 2
        sl = slice(g * O, (g + 1) * O)
        in_maps.append(
            {
                "xqt": tile_x(q[b].T),
                "xkvt": tile_x(kv[b].T),
                "wqt": tile_w(W_q[sl, :].T),
                "wkt": tile_w(W_k[sl, :].T),
                "wvt": tile_w(W_v[sl, :].T),
                "wot": np.ascontiguousarray(
                    W_o[:, sl].T.reshape(O // 128, 128, D).transpose(1, 0, 2)
                ),
                "ones": ones,
            }
        )
    return in_maps


def assemble_output(results):
    out = np.empty((4, S, D), dtype=np.float32)
    for b in range(4):
        partial = results[2 * b]["otp"].astype(np.float32) + results[
            2 * b + 1
        ]["otp"].astype(np.float32)  # [D, S]
        out[b] = partial.T
    return out


_NC_CACHE = None


def kernel(**inputs) -> np.ndarray:
    global _NC_CACHE
    from concourse.bass_utils import run_bass_kernel_spmd

    if _NC_CACHE is None:
        _NC_CACHE = build_kernel()
    in_maps = make_in_maps(
        inputs["query_input"],
        inputs["kv_input"],
        inputs["W_q"],
        inputs["W_k"],
        inputs["W_v"],
        inputs["W_o"],
    )
    res = run_bass_kernel_spmd(_NC_CACHE, in_maps, list(range(8)))
    return assemble_output(res.results)


# revision 27
# speedup vs baseline: 1.1879x; 1.1879x over previous
"""Multi-head attention (B=4, S=2048, D=1024, H=16, Dh=64) on 8 TRN2 cores.

Sharding: data-parallel over batch (4) x tensor-parallel over heads (2 groups
of 8). Core c handles batch c//2, head-group c%2 (heads g*8..g*8+7 via the
column split of W_q/W_k/W_v and row split of W_o). Each core emits a partial
output projection o^T [1024, 2048] bf16; the host sums the two head-group
partials per batch in f32 and transposes back.

All matmuls bf16 (inputs pre-cast on host), fp32 PSUM accumulation:
  Phase A (per head-pair ot): K^T, Q^T, V slices via bf16 matmuls. Chains run
  as PAIRS across psAO's two buffers, it-major interleaved, so each
  LDWEIGHTS hides under the partner chain's matmul stream.
  Phase B (per q-chunk qc, head-pair j):
    S^T tiles = K^T.T Q^T   (row-tiled head pairs at (0,0)/(64,0), K=64)
    P = exp(S/8)            (ACT per-ktile, PSUM[128,2,512] -> bf16)
    ctx^T += V.T P^T        (col-tiled pairs at (0,0)/(0,64), M=64)
    sums: DVE bf16 quarter-tree + col-tiled ones matmuls into one PSUM bank
    ctx^T *= 1/sums         (reciprocal_approx_fast + fused PSUM mul)
  Phase C: o^T = Wo_g^T.T @ ctx^T -> bf16 -> DMA out.

PSUM budget (8 banks): psS 2x2 + psC 1 + psR 1 + psAO 2 (projections and
output projection share a pool so phases overlap without false deps).
"""

import numpy as np

import concourse.bacc as bacc
import concourse.mybir as mybir
import concourse.tile as tile

D = 1024  # model dim
S = 2048  # sequence length
O = 512  # per-core projected dim (8 heads x 64)
IT = D // 128  # 8 input-dim tiles
NP = 4  # head pairs per core
QC = S // 512  # 4 q-chunks
KT = S // 128  # 16 k-tiles
VT = S // 128  # 16 s-tiles of V
MT = D // 128  # 8 output m-tiles
SCALE = 0.125  # 1/sqrt(64)

F32 = mybir.dt.float32
BF16 = mybir.dt.bfloat16


def build_kernel():
    nc = bacc.Bacc("TRN2", target_bir_lowering=False, debug=False, num_devices=8)
    xqt = nc.declare_dram_parameter("xqt", [QC, 128, IT, 512], BF16, isOutput=False)
    xkvt = nc.declare_dram_parameter("xkvt", [QC, 128, IT, 512], BF16, isOutput=False)
    wqt = nc.declare_dram_parameter("wqt", [128, IT, O], BF16, isOutput=False)
    wkt = nc.declare_dram_parameter("wkt", [128, IT, O], BF16, isOutput=False)
    wvt = nc.declare_dram_parameter("wvt", [128, IT, O], BF16, isOutput=False)
    wot = nc.declare_dram_parameter("wot", [128, O // 128, D], BF16, isOutput=False)
    ones = nc.declare_dram_parameter("ones", [128, 512], BF16, isOutput=False)
    otp = nc.declare_dram_parameter("otp", [D, S], BF16, isOutput=True)

    with tile.TileContext(nc) as tc:
        with (
            tc.tile_pool(name="persist", bufs=1) as persist,
            tc.tile_pool(name="ctxp", bufs=1) as ctxp,
            tc.tile_pool(name="pp", bufs=3) as ppp,
            tc.tile_pool(name="sump", bufs=2) as sump,
            tc.tile_pool(name="rp", bufs=2) as rp,
            tc.tile_pool(name="ots", bufs=3) as ots,
            tc.tile_pool(name="psS", bufs=2, space="PSUM") as psS,
            tc.tile_pool(name="psC", bufs=1, space="PSUM") as psC,
            tc.tile_pool(name="psR", bufs=1, space="PSUM") as psR,
            tc.tile_pool(name="psAO", bufs=2, space="PSUM") as psAO,
        ):
            qt_chunks = [
                [persist.tile([128, 512], BF16, name=f"qtc{j}_{c}") for c in range(QC)]
                for j in range(NP)
            ]
            kt_chunks = [
                [persist.tile([128, 512], BF16, name=f"ktc{j}_{c}") for c in range(QC)]
                for j in range(NP)
            ]
            v_tiles = [persist.tile([128, O], BF16, name=f"vt{t}") for t in range(VT)]
            wo_sb = persist.tile([128, O // 128, D], BF16)
            ones_full = persist.tile([128, 512], BF16)
            nc.scalar.dma_start(out=ones_full, in_=ones[:, :])
            ones_bf = ones_full[:, 0:64]

            # PE warm-up filler: lowest-priority dummy matmuls the scheduler
            # slots into PE-idle gaps (DMA head) to keep the HAM clock warm.
            warm_ps = psR.tile([128, 512], F32, tag="psR", name="warm_ps")
            with tc.high_priority(offset=-(10**6)):
                for _ in range(16):
                    nc.tensor.matmul(
                        warm_ps[0:64, :],
                        ones_bf,
                        ones_full,
                        start=True,
                        stop=True,
                    )

            # ---- Phase A: projections (overlaps phase B via disjoint pools) ----
            with tc.tile_pool(name="wqx", bufs=1) as wqp:
                wq_sb = wqp.tile([128, IT, O], BF16)
                wk_sb = wqp.tile([128, IT, O], BF16)
                wv_sb = wqp.tile([128, IT, O], BF16)
                # All input DMAs on the sync HW-DGE queue, in the order the
                # projection pipeline consumes them (K first, then Q, V, O).
                # First tiles split in halves so the first proj chain starts
                # before its full tile lands.
                xq_c, xkv_c = [], []
                # wk + wq ride the scalar engine's HW-DGE ring (idle until
                # the first exp), halving the 4MB critical head with the
                # activations streaming on the sync ring concurrently.
                nc.scalar.dma_start(out=wk_sb, in_=wkt[:, :, :])
                nc.scalar.dma_start(out=wq_sb, in_=wqt[:, :, :])
                xkv_1 = wqp.tile([128, IT, 512], BF16, name="xkv0")
                for h in range(4):
                    sl = slice(2 * h, 2 * h + 2)
                    nc.sync.dma_start(out=xkv_1[:, sl, :], in_=xkvt[0, :, sl, :])
                xkv_c.append(xkv_1)
                xq_1 = wqp.tile([128, IT, 512], BF16, name="xq0")
                for h in range(2):
                    sl = slice(4 * h, 4 * h + 4)
                    nc.sync.dma_start(out=xq_1[:, sl, :], in_=xqt[0, :, sl, :])
                xq_c.append(xq_1)
                for c in range(1, QC):
                    xkv_1 = wqp.tile([128, IT, 512], BF16, name=f"xkv{c}")
                    nc.sync.dma_start(out=xkv_1, in_=xkvt[c, :, :, :])
                    xkv_c.append(xkv_1)
                nc.sync.dma_start(out=wv_sb, in_=wvt[:, :, :])
                for c in range(1, QC):
                    xq_1 = wqp.tile([128, IT, 512], BF16, name=f"xq{c}")
                    nc.sync.dma_start(out=xq_1, in_=xqt[c, :, :, :])
                    xq_c.append(xq_1)
                nc.sync.dma_start(out=wo_sb, in_=wot[:, :, :])

                # ---- projection chain pairs: two psum chains (psAO bufs=2),
                # it-major interleaved so each LDW hides under the partner
                # chain's matmul stream. specs: (dst, 'kq'|'v', w/x args) ----
                _pn = [0]

                def proj_pair(specs):
                    pss = []
                    for _ in specs:
                        _pn[0] += 1
                        pss.append(
                            psAO.tile(
                                [128, 512], F32, tag="psAO", name=f"pj{_pn[0]}"
                            )
                        )
                    for it in range(IT):
                        for ps, spec in zip(pss, specs):
                            if spec[1] == "kq":
                                dst, _, w_sb, x_tile, ot = spec
                                nc.tensor.matmul(
                                    ps,
                                    w_sb[:, it, ot * 128 : (ot + 1) * 128],
                                    x_tile[:, it, :],
                                    start=(it == 0),
                                    stop=(it == IT - 1),
                                )
                            else:
                                dst, _, vt = spec
                                nc.tensor.matmul(
                                    ps,
                                    xkv_c[vt // 4][
                                        :, it, (vt % 4) * 128 : (vt % 4 + 1) * 128
                                    ],
                                    wv_sb[:, it, :],
                                    start=(it == 0),
                                    stop=(it == IT - 1),
                                )
                    for ps, spec in zip(pss, specs):
                        nc.vector.tensor_copy(spec[0], ps)

                def K(j, c):
                    return (kt_chunks[j][c], "kq", wk_sb, xkv_c[c], j)

                def Q(j, c):
                    return (qt_chunks[j][c], "kq", wq_sb, xq_c[c], j)

                def V(vt):
                    return (v_tiles[vt], "v", vt)

                # Head-pair 0's K/Q first so attention (and ACT) starts early;
                # V next (PV consumes v_tiles in kt order); rest of K/Q after.
                # Q chunks 2,3 are deferred into the attention phase (emitted
                # right before unit (2,0)) to offload the oversubscribed head.
                proj_pair([K(0, 0)])
                proj_pair([Q(0, 0)])
                proj_pair([K(0, 1), K(0, 2)])
                proj_pair([K(0, 3), Q(0, 1)])
                for vt in range(0, VT // 2, 2):
                    proj_pair([V(vt), V(vt + 1)])
                proj_pair([K(1, 0), K(1, 1)])
                proj_pair([K(1, 2), K(1, 3)])
                proj_pair([Q(1, 0), Q(1, 1)])
                for vt in range(VT // 2, VT, 2):
                    proj_pair([V(vt), V(vt + 1)])
                # K/Q for head-pairs 2,3 are interleaved into the first
                # attention units below: their consumers (units (qc,2),(qc,3))
                # run 50us+ later, safely past the LDWEIGHTS pull-ahead
                # window, and this unloads the early-phase PE crunch.

                # ---- Phase B: attention (pair-outer); Phase C: output proj ----
                ctx_tiles = [
                    ctxp.tile([128, NP, 512], BF16, name=f"ctx{c}") for c in range(QC)
                ]
                # First qc pair interleaved j-major (buys the projection
                # pipeline runway); second half qc-major. out_proj(qc) is
                # emitted as soon as ctx(qc) is complete so its chains fill
                # the ACT-bound attention phase's PE slack, not the tail.
                def out_proj(qc, tail=False):
                    qsl = slice(qc * 512, (qc + 1) * 512)
                    for mt in range(MT):
                        ps_o = psAO.tile(
                            [128, 512], F32, tag="psAO", name=f"ps_o{qc}_{mt}"
                        )
                        for jt in range(NP):
                            nc.tensor.matmul(
                                ps_o,
                                wo_sb[:, jt, mt * 128 : (mt + 1) * 128],
                                ctx_tiles[qc][:, jt, :],
                                start=(jt == 0),
                                stop=(jt == NP - 1),
                            )
                        ot_sb = ots.tile([128, 512], BF16, tag="ot")
                        if tail and mt % 2 == 1:
                            # Tail copies alternate onto the (now idle)
                            # scalar engine so psAO slots free twice as fast.
                            nc.scalar.activation(
                                out=ot_sb,
                                in_=ps_o,
                                func=mybir.ActivationFunctionType.Copy,
                            )
                        else:
                            nc.vector.tensor_copy(ot_sb, ps_o)
                        nc.sync.dma_start(
                            out=otp[mt * 128 : (mt + 1) * 128, qsl], in_=ot_sb
                        )

                unit_order = [(qc, j) for j in range(NP) for qc in (0, 1)]
                unit_order += [(qc, j) for qc in (2, 3) for j in range(NP)]
                for qc, j in unit_order:
                    if (qc, j) == (1, 0):
                        proj_pair([K(2, 0), K(2, 1)])
                        proj_pair([K(2, 2), K(2, 3)])
                    if (qc, j) == (0, 1):
                        proj_pair([Q(2, 0), Q(2, 1)])
                        proj_pair([K(3, 0), K(3, 1)])
                    if (qc, j) == (1, 1):
                        proj_pair([K(3, 2), K(3, 3)])
                        proj_pair([Q(3, 0), Q(3, 1)])
                    if (qc, j) == (2, 0):
                        for ot in range(NP):
                            proj_pair([Q(ot, 2), Q(ot, 3)])
                    if (qc, j) == (2, 1):
                        out_proj(0)
                    if (qc, j) == (2, 3):
                        out_proj(1)
                    if (qc, j) == (3, 1):
                        out_proj(2)
                    if True:
                        psum_ctx = psC.tile([128, 512], F32, tag="psC")
                        psum_r = psR.tile([128, 512], F32, tag="psR")
                        s1_tiles = []
                        sh_tiles = []
                        for qi in range(4):  # quarters of the k range
                            pq = ppp.tile([128, 4, 2, 512], BF16, tag="pp")
                            for ki in range(4):  # per-ktile ACT batches
                                kt = qi * 4 + ki
                                ps_s = psS.tile([128, 2, 512], F32, tag="psS")
                                with tc.high_priority(offset=50000):
                                    nc.tensor.matmul(
                                        ps_s[:, 0, :],
                                        kt_chunks[j][kt // 4][0:64, (kt % 4) * 128 : (kt % 4 + 1) * 128],
                                        qt_chunks[j][qc][0:64, :],
                                        start=True,
                                        stop=True,
                                        tile_position=(0, 0),
                                    )
                                    nc.tensor.matmul(
                                        ps_s[:, 1, :],
                                        kt_chunks[j][kt // 4][64:128, (kt % 4) * 128 : (kt % 4 + 1) * 128],
                                        qt_chunks[j][qc][64:128, :],
                                        start=True,
                                        stop=True,
                                        tile_position=(64, 0),
                                    )
                                    nc.scalar.activation(
                                        out=pq[:, ki, :, :],
                                        in_=ps_s[:, :, :],
                                        func=mybir.ActivationFunctionType.Exp,
                                        scale=SCALE,
                                    )
                            # PV + sums for this quarter
                            with tc.high_priority(offset=50000):
                                for ki in range(4):
                                    kt = qi * 4 + ki
                                    first = kt == 0
                                    last = kt == KT - 1
                                    nc.tensor.matmul(
                                        psum_ctx[0:64, :],
                                        v_tiles[kt][:, j * 128 : j * 128 + 64],
                                        pq[:, ki, 0, :],
                                        start=first,
                                        stop=last,
                                        tile_position=(0, 0),
                                    )
                                    nc.tensor.matmul(
                                        psum_ctx[64:128, :],
                                        v_tiles[kt][:, j * 128 + 64 : (j + 1) * 128],
                                        pq[:, ki, 1, :],
                                        start=first,
                                        stop=last,
                                        tile_position=(0, 64),
                                    )
                                tq = sump.tile([128, 2, 2, 512], BF16, tag="tq", bufs=1)
                                s1 = sump.tile([128, 2, 512], BF16, tag="s1")
                                with nc.allow_low_precision(reason="softmax sum partials"):
                                    nc.vector.tensor_add(
                                        tq, pq[:, 0:2, :, :], pq[:, 2:4, :, :]
                                    )
                                    nc.vector.tensor_add(
                                        s1, tq[:, 0, :, :], tq[:, 1, :, :]
                                    )
                                s1_tiles.append(s1)
                                if qi % 2 == 1:
                                    sh = sump.tile([128, 2, 512], BF16, tag="sh")
                                    with nc.allow_low_precision(reason="softmax sum partials"):
                                        nc.vector.tensor_add(
                                            sh, s1_tiles[qi - 1], s1_tiles[qi]
                                        )
                                    sh_tiles.append(sh)
                        s_all = sh_tiles[0]
                        with nc.allow_low_precision(reason="softmax sum partials"):
                            nc.vector.tensor_add(s_all, sh_tiles[0], sh_tiles[1])
                        with tc.high_priority(offset=50000):
                            nc.tensor.matmul(
                                psum_r[0:64, :],
                                ones_bf,
                                s_all[:, 0, :],
                                start=True,
                                stop=True,
                                tile_position=(0, 0),
                            )
                            nc.tensor.matmul(
                                psum_r[64:128, :],
                                ones_bf,
                                s_all[:, 1, :],
                                start=True,
                                stop=True,
                                tile_position=(0, 64),
                            )
                        # normalize
                        with tc.high_priority(offset=50000):
                            r_tile = rp.tile([128, 512], F32, tag="r")
                            nc.vector.reciprocal_approx_fast(out=r_tile, in_=psum_r)
                            with nc.allow_low_precision(reason="bf16 ctx for PE"):
                                nc.vector.tensor_mul(
                                    ctx_tiles[qc][:, j, :], psum_ctx, r_tile
                                )
                # Phase C tail: last q-chunk's output projection
                out_proj(3, tail=True)
    nc.compile()
    return nc


def make_in_maps(query_input, kv_input, W_q, W_k, W_v, W_o):
    import ml_dtypes

    bf16 = ml_dtypes.bfloat16
    q = np.asarray(query_input, dtype=np.float32).astype(bf16)
    kv = np.asarray(kv_input, dtype=np.float32).astype(bf16)
    W_q = np.asarray(W_q, dtype=np.float32).astype(bf16)
    W_k = np.asarray(W_k, dtype=np.float32).astype(bf16)
    W_v = np.asarray(W_v, dtype=np.float32).astype(bf16)
    W_o = np.asarray(W_o, dtype=np.float32).astype(bf16)
    ones = np.ones((128, 512), dtype=bf16)

    def tile_x(xt):  # [D, S] -> [QC, 128, IT, 512]
        return np.ascontiguousarray(
            xt.reshape(IT, 128, QC, 512).transpose(2, 1, 0, 3)
        )

    def tile_w(wt):  # [D, O] -> [128, IT, O]
        return np.ascontiguousarray(wt.reshape(IT, 128, O).transpose(1, 0, 2))

    in_maps = []
    for c in range(8):
        b, g = c // 2, c % 2
        sl = slice(g * O, (g + 1) * O)
        in_maps.append(
            {
                "xqt": tile_x(q[b].T),
                "xkvt": tile_x(kv[b].T),
                "wqt": tile_w(W_q[sl, :].T),
                "wkt": tile_w(W_k[sl, :].T),
                "wvt": tile_w(W_v[sl, :].T),
                "wot": np.ascontiguousarray(
                    W_o[:, sl].T.reshape(O // 128, 128, D).transpose(1, 0, 2)
                ),
                "ones": ones,
            }
        )
    return in_maps


def assemble_output(results):
    out = np.empty((4, S, D), dtype=np.float32)
    for b in range(4):
        partial = results[2 * b]["otp"].astype(np.float32) + results[
            2 * b + 1
        ]["otp"].astype(np.float32)  # [D, S]
        out[b] = partial.T
    return out


_NC_CACHE = None


def kernel(**inputs) -> np.ndarray:
    global _NC_CACHE
    from concourse.bass_utils import run_bass_kernel_spmd

    if _NC_CACHE is None:
        _NC_CACHE = build_kernel()
    in_maps = make_in_maps(
        inputs["query_input"],
        inputs["kv_input"],
        inputs["W_q"],
        inputs["W_k"],
        inputs["W_v"],
        inputs["W_o"],
    )
    res = run_bass_kernel_spmd(_NC_CACHE, in_maps, list(range(8)))
    return assemble_output(res.results)


# revision 30
# speedup vs baseline: 1.1981x; 1.0086x over previous
"""Multi-head attention (B=4, S=2048, D=1024, H=16, Dh=64) on 8 TRN2 cores.

Sharding: data-parallel over batch (4) x tensor-parallel over heads (2 groups
of 8). Core c handles batch c//2, head-group c%2 (heads g*8..g*8+7 via the
column split of W_q/W_k/W_v and row split of W_o). Each core emits a partial
output projection o^T [1024, 2048] bf16; the host sums the two head-group
partials per batch in f32 and transposes back.

All matmuls bf16 (inputs pre-cast on host), fp32 PSUM accumulation:
  Phase A (per head-pair ot): K^T, Q^T, V slices via bf16 matmuls. Chains run
  as PAIRS across psAO's two buffers, it-major interleaved, so each
  LDWEIGHTS hides under the partner chain's matmul stream.
  Phase B (per q-chunk qc, head-pair j):
    S^T tiles = K^T.T Q^T   (row-tiled head pairs at (0,0)/(64,0), K=64)
    P = exp(S/8)            (ACT per-ktile, PSUM[128,2,512] -> bf16)
    ctx^T += V.T P^T        (col-tiled pairs at (0,0)/(0,64), M=64)
    sums: DVE bf16 quarter-tree + col-tiled ones matmuls into one PSUM bank
    ctx^T *= 1/sums         (reciprocal_approx_fast + fused PSUM mul)
  Phase C: o^T = Wo_g^T.T @ ctx^T -> bf16 -> DMA out.

PSUM budget (8 banks): psS 2x2 + psC 1 + psR 1 + psAO 2 (projections and
output projection share a pool so phases overlap without false deps).
"""

import numpy as np

import concourse.bacc as bacc
import concourse.mybir as mybir
import concourse.tile as tile

D = 1024  # model dim
S = 2048  # sequence length
O = 512  # per-core projected dim (8 heads x 64)
IT = D // 128  # 8 input-dim tiles
NP = 4  # head pairs per core
QC = S // 512  # 4 q-chunks
KT = S // 128  # 16 k-tiles
VT = S // 128  # 16 s-tiles of V
MT = D // 128  # 8 output m-tiles
SCALE = 0.125  # 1/sqrt(64)

F32 = mybir.dt.float32
BF16 = mybir.dt.bfloat16


def build_kernel():
    nc = bacc.Bacc("TRN2", target_bir_lowering=False, debug=False, num_devices=8)
    xqt = nc.declare_dram_parameter("xqt", [QC, 128, IT, 512], BF16, isOutput=False)
    xkvt = nc.declare_dram_parameter("xkvt", [QC, 128, IT, 512], BF16, isOutput=False)
    wqt = nc.declare_dram_parameter("wqt", [128, IT, O], BF16, isOutput=False)
    wkt = nc.declare_dram_parameter("wkt", [128, IT, O], BF16, isOutput=False)
    wvt = nc.declare_dram_parameter("wvt", [128, IT, O], BF16, isOutput=False)
    wot = nc.declare_dram_parameter("wot", [128, O // 128, D], BF16, isOutput=False)
    ones = nc.declare_dram_parameter("ones", [128, 512], BF16, isOutput=False)
    otp = nc.declare_dram_parameter("otp", [D, S], BF16, isOutput=True)

    with tile.TileContext(nc) as tc:
        with (
            tc.tile_pool(name="persist", bufs=1) as persist,
            tc.tile_pool(name="ctxp", bufs=1) as ctxp,
            tc.tile_pool(name="pp", bufs=3) as ppp,
            tc.tile_pool(name="sump", bufs=2) as sump,
            tc.tile_pool(name="rp", bufs=2) as rp,
            tc.tile_pool(name="ots", bufs=3) as ots,
            tc.tile_pool(name="psS", bufs=2, space="PSUM") as psS,
            tc.tile_pool(name="psC", bufs=1, space="PSUM") as psC,
            tc.tile_pool(name="psR", bufs=1, space="PSUM") as psR,
            tc.tile_pool(name="psAO", bufs=2, space="PSUM") as psAO,
        ):
            qt_chunks = [
                [persist.tile([128, 512], BF16, name=f"qtc{j}_{c}") for c in range(QC)]
                for j in range(NP)
            ]
            kt_chunks = [
                [persist.tile([128, 512], BF16, name=f"ktc{j}_{c}") for c in range(QC)]
                for j in range(NP)
            ]
            v_tiles = [persist.tile([128, O], BF16, name=f"vt{t}") for t in range(VT)]
            wo_sb = persist.tile([128, O // 128, D], BF16)
            ones_full = persist.tile([128, 512], BF16)
            nc.scalar.dma_start(out=ones_full, in_=ones[:, :])
            ones_bf = ones_full[:, 0:64]

            # PE warm-up filler: lowest-priority dummy matmuls the scheduler
            # slots into PE-idle gaps (DMA head) to keep the HAM clock warm.
            warm_ps = psR.tile([128, 512], F32, tag="psR", name="warm_ps")
            with tc.high_priority(offset=-(10**6)):
                for _ in range(16):
                    nc.tensor.matmul(
                        warm_ps[0:64, :],
                        ones_bf,
                        ones_full,
                        start=True,
                        stop=True,
                    )

            # ---- Phase A: projections (overlaps phase B via disjoint pools) ----
            with tc.tile_pool(name="wqx", bufs=1) as wqp:
                wq_sb = wqp.tile([128, IT, O], BF16)
                wk_sb = wqp.tile([128, IT, O], BF16)
                wv_sb = wqp.tile([128, IT, O], BF16)
                # All input DMAs on the sync HW-DGE queue, in the order the
                # projection pipeline consumes them (K first, then Q, V, O).
                # First tiles split in halves so the first proj chain starts
                # before its full tile lands.
                xq_c, xkv_c = [], []
                # wk + wq ride the scalar engine's HW-DGE ring (idle until
                # the first exp), halving the 4MB critical head with the
                # activations streaming on the sync ring concurrently.
                nc.scalar.dma_start(out=wk_sb, in_=wkt[:, :, :])
                nc.scalar.dma_start(out=wq_sb, in_=wqt[:, :, :])
                xkv_1 = wqp.tile([128, IT, 512], BF16, name="xkv0")
                for h in range(4):
                    sl = slice(2 * h, 2 * h + 2)
                    nc.sync.dma_start(out=xkv_1[:, sl, :], in_=xkvt[0, :, sl, :])
                xkv_c.append(xkv_1)
                xq_1 = wqp.tile([128, IT, 512], BF16, name="xq0")
                for h in range(2):
                    sl = slice(4 * h, 4 * h + 4)
                    nc.sync.dma_start(out=xq_1[:, sl, :], in_=xqt[0, :, sl, :])
                xq_c.append(xq_1)
                for c in range(1, QC):
                    xkv_1 = wqp.tile([128, IT, 512], BF16, name=f"xkv{c}")
                    nc.sync.dma_start(out=xkv_1, in_=xkvt[c, :, :, :])
                    xkv_c.append(xkv_1)
                nc.sync.dma_start(out=wv_sb, in_=wvt[:, :, :])
                for c in range(1, QC):
                    xq_1 = wqp.tile([128, IT, 512], BF16, name=f"xq{c}")
                    nc.sync.dma_start(out=xq_1, in_=xqt[c, :, :, :])
                    xq_c.append(xq_1)
                nc.sync.dma_start(out=wo_sb, in_=wot[:, :, :])

                # ---- projection chain pairs: two psum chains (psAO bufs=2),
                # it-major interleaved so each LDW hides under the partner
                # chain's matmul stream. specs: (dst, 'kq'|'v', w/x args) ----
                _pn = [0]

                def proj_pair(specs):
                    pss = []
                    for _ in specs:
                        _pn[0] += 1
                        pss.append(
                            psAO.tile(
                                [128, 512], F32, tag="psAO", name=f"pj{_pn[0]}"
                            )
                        )
                    for it in range(IT):
                        for ps, spec in zip(pss, specs):
                            if spec[1] == "kq":
                                dst, _, w_sb, x_tile, ot = spec
                                nc.tensor.matmul(
                                    ps,
                                    w_sb[:, it, ot * 128 : (ot + 1) * 128],
                                    x_tile[:, it, :],
                                    start=(it == 0),
                                    stop=(it == IT - 1),
                                )
                            else:
                                dst, _, vt = spec
                                nc.tensor.matmul(
                                    ps,
                                    xkv_c[vt // 4][
                                        :, it, (vt % 4) * 128 : (vt % 4 + 1) * 128
                                    ],
                                    wv_sb[:, it, :],
                                    start=(it == 0),
                                    stop=(it == IT - 1),
                                )
                    for ps, spec in zip(pss, specs):
                        nc.vector.tensor_copy(spec[0], ps)

                def K(j, c):
                    return (kt_chunks[j][c], "kq", wk_sb, xkv_c[c], j)

                def Q(j, c):
                    return (qt_chunks[j][c], "kq", wq_sb, xq_c[c], j)

                def V(vt):
                    return (v_tiles[vt], "v", vt)

                # Head-pair 0's K/Q first so attention (and ACT) starts early;
                # V next (PV consumes v_tiles in kt order); rest of K/Q after.
                # Q chunks 2,3 are deferred into the attention phase (emitted
                # right before unit (2,0)) to offload the oversubscribed head.
                proj_pair([K(0, 0), Q(0, 0)])
                proj_pair([K(0, 1), K(0, 2)])
                proj_pair([K(0, 3), Q(0, 1)])
                for vt in range(0, VT // 2, 2):
                    proj_pair([V(vt), V(vt + 1)])
                proj_pair([K(1, 0), K(1, 1)])
                proj_pair([K(1, 2), K(1, 3)])
                proj_pair([Q(1, 0), Q(1, 1)])
                for vt in range(VT // 2, VT, 2):
                    proj_pair([V(vt), V(vt + 1)])
                # K/Q for head-pairs 2,3 are interleaved into the first
                # attention units below: their consumers (units (qc,2),(qc,3))
                # run 50us+ later, safely past the LDWEIGHTS pull-ahead
                # window, and this unloads the early-phase PE crunch.

                # ---- Phase B: attention (pair-outer); Phase C: output proj ----
                ctx_tiles = [
                    ctxp.tile([128, NP, 512], BF16, name=f"ctx{c}") for c in range(QC)
                ]
                # First qc pair interleaved j-major (buys the projection
                # pipeline runway); second half qc-major. out_proj(qc) is
                # emitted as soon as ctx(qc) is complete so its chains fill
                # the ACT-bound attention phase's PE slack, not the tail.
                def out_proj(qc, tail=False):
                    qsl = slice(qc * 512, (qc + 1) * 512)
                    for mt in range(MT):
                        ps_o = psAO.tile(
                            [128, 512], F32, tag="psAO", name=f"ps_o{qc}_{mt}"
                        )
                        for jt in range(NP):
                            nc.tensor.matmul(
                                ps_o,
                                wo_sb[:, jt, mt * 128 : (mt + 1) * 128],
                                ctx_tiles[qc][:, jt, :],
                                start=(jt == 0),
                                stop=(jt == NP - 1),
                            )
                        ot_sb = ots.tile([128, 512], BF16, tag="ot")
                        if tail and mt % 2 == 1:
                            # Tail copies alternate onto the (now idle)
                            # scalar engine so psAO slots free twice as fast.
                            nc.scalar.activation(
                                out=ot_sb,
                                in_=ps_o,
                                func=mybir.ActivationFunctionType.Copy,
                            )
                        else:
                            nc.vector.tensor_copy(ot_sb, ps_o)
                        nc.sync.dma_start(
                            out=otp[mt * 128 : (mt + 1) * 128, qsl], in_=ot_sb
                        )

                unit_order = [(qc, j) for j in range(NP) for qc in (0, 1)]
                unit_order += [(qc, j) for qc in (2, 3) for j in range(NP)]
                for ui, (qc, j) in enumerate(unit_order):
                    if (qc, j) == (1, 0):
                        proj_pair([K(2, 0), K(2, 1)])
                        proj_pair([K(2, 2), K(2, 3)])
                    if (qc, j) == (0, 1):
                        proj_pair([Q(2, 0), Q(2, 1)])
                        proj_pair([K(3, 0), K(3, 1)])
                    if (qc, j) == (1, 1):
                        proj_pair([K(3, 2), K(3, 3)])
                        proj_pair([Q(3, 0), Q(3, 1)])
                    if (qc, j) == (2, 0):
                        for ot in range(NP):
                            proj_pair([Q(ot, 2), Q(ot, 3)])
                    if (qc, j) == (2, 1):
                        out_proj(0)
                    if (qc, j) == (2, 3):
                        out_proj(1)
                    if (qc, j) == (3, 1):
                        out_proj(2)
                    if True:
                        # Alternate the ctx/sums banks per unit so unit n+1's
                        # PV chain doesn't wait on unit n's normalize read.
                        cp_, rp_ = (psC, psR) if ui % 2 == 0 else (psR, psC)
                        psum_ctx = cp_.tile(
                            [128, 512], F32, tag="psC" if ui % 2 == 0 else "psR"
                        )
                        psum_r = rp_.tile(
                            [128, 512], F32, tag="psR" if ui % 2 == 0 else "psC"
                        )
                        s1_tiles = []
                        sh_tiles = []
                        for qi in range(4):  # quarters of the k range
                            pq = ppp.tile([128, 4, 2, 512], BF16, tag="pp")
                            for ki in range(4):  # per-ktile ACT batches
                                kt = qi * 4 + ki
                                ps_s = psS.tile([128, 2, 512], F32, tag="psS")
                                with tc.high_priority(offset=50000):
                                    nc.tensor.matmul(
                                        ps_s[:, 0, :],
                                        kt_chunks[j][kt // 4][0:64, (kt % 4) * 128 : (kt % 4 + 1) * 128],
                                        qt_chunks[j][qc][0:64, :],
                                        start=True,
                                        stop=True,
                                        tile_position=(0, 0),
                                    )
                                    nc.tensor.matmul(
                                        ps_s[:, 1, :],
                                        kt_chunks[j][kt // 4][64:128, (kt % 4) * 128 : (kt % 4 + 1) * 128],
                                        qt_chunks[j][qc][64:128, :],
                                        start=True,
                                        stop=True,
                                        tile_position=(64, 0),
                                    )
                                    nc.scalar.activation(
                                        out=pq[:, ki, :, :],
                                        in_=ps_s[:, :, :],
                                        func=mybir.ActivationFunctionType.Exp,
                                        scale=SCALE,
                                    )
                            # PV + sums for this quarter
                            with tc.high_priority(offset=50000):
                                for ki in range(4):
                                    kt = qi * 4 + ki
                                    first = kt == 0
                                    last = kt == KT - 1
                                    nc.tensor.matmul(
                                        psum_ctx[0:64, :],
                                        v_tiles[kt][:, j * 128 : j * 128 + 64],
                                        pq[:, ki, 0, :],
                                        start=first,
                                        stop=last,
                                        tile_position=(0, 0),
                                    )
                                    nc.tensor.matmul(
                                        psum_ctx[64:128, :],
                                        v_tiles[kt][:, j * 128 + 64 : (j + 1) * 128],
                                        pq[:, ki, 1, :],
                                        start=first,
                                        stop=last,
                                        tile_position=(0, 64),
                                    )
                                tq = sump.tile([128, 2, 2, 512], BF16, tag="tq", bufs=1)
                                s1 = sump.tile([128, 2, 512], BF16, tag="s1")
                                with nc.allow_low_precision(reason="softmax sum partials"):
                                    nc.vector.tensor_add(
                                        tq, pq[:, 0:2, :, :], pq[:, 2:4, :, :]
                                    )
                                    nc.vector.tensor_add(
                                        s1, tq[:, 0, :, :], tq[:, 1, :, :]
                                    )
                                s1_tiles.append(s1)
                                if qi % 2 == 1:
                                    sh = sump.tile([128, 2, 512], BF16, tag="sh")
                                    with nc.allow_low_precision(reason="softmax sum partials"):
                                        nc.vector.tensor_add(
                                            sh, s1_tiles[qi - 1], s1_tiles[qi]
                                        )
                                    sh_tiles.append(sh)
                        s_all = sh_tiles[0]
                        with nc.allow_low_precision(reason="softmax sum partials"):
                            nc.vector.tensor_add(s_all, sh_tiles[0], sh_tiles[1])
                        with tc.high_priority(offset=50000):
                            nc.tensor.matmul(
                                psum_r[0:64, :],
                                ones_bf,
                                s_all[:, 0, :],
                                start=True,
                                stop=True,
                                tile_position=(0, 0),
                            )
                            nc.tensor.matmul(
                                psum_r[64:128, :],
                                ones_bf,
                                s_all[:, 1, :],
                                start=True,
                                stop=True,
                                tile_position=(0, 64),
                            )
                        # normalize
                        with tc.high_priority(offset=50000):
                            r_tile = rp.tile([128, 512], F32, tag="r")
                            nc.vector.reciprocal_approx_fast(out=r_tile, in_=psum_r)
                            with nc.allow_low_precision(reason="bf16 ctx for PE"):
                                nc.vector.tensor_mul(
                                    ctx_tiles[qc][:, j, :], psum_ctx, r_tile
                                )
                # Phase C tail: last q-chunk's output projection
                out_proj(3, tail=True)
    nc.compile()
    return nc


def make_in_maps(query_input, kv_input, W_q, W_k, W_v, W_o):
    import ml_dtypes

    bf16 = ml_dtypes.bfloat16
    q = np.asarray(query_input, dtype=np.float32).astype(bf16)
    kv = np.asarray(kv_input, dtype=np.float32).astype(bf16)
    W_q = np.asarray(W_q, dtype=np.float32).astype(bf16)
    W_k = np.asarray(W_k, dtype=np.float32).astype(bf16)
    W_v = np.asarray(W_v, dtype=np.float32).astype(bf16)
    W_o = np.asarray(W_o, dtype=np.float32).astype(bf16)
    ones = np.ones((128, 512), dtype=bf16)

    def tile_x(xt):  # [D, S] -> [QC, 128, IT, 512]
        return np.ascontiguousarray(
            xt.reshape(IT, 128, QC, 512).transpose(2, 1, 0, 3)
        )

    def tile_w(wt):  # [D, O] -> [128, IT, O]
        return np.ascontiguousarray(wt.reshape(IT, 128, O).transpose(1, 0, 2))

    in_maps = []
    for c in range(8):
        b, g = c // 2, c % 2
        sl = slice(g * O, (g + 1) * O)
        in_maps.append(
            {
                "xqt": tile_x(q[b].T),
                "xkvt": tile_x(kv[b].T),
                "wqt": tile_w(W_q[sl, :].T),
                "wkt": tile_w(W_k[sl, :].T),
                "wvt": tile_w(W_v[sl, :].T),
                "wot": np.ascontiguousarray(
                    W_o[:, sl].T.reshape(O // 128, 128, D).transpose(1, 0, 2)
                ),
                "ones": ones,
            }
        )
    return in_maps


def assemble_output(results):
    out = np.empty((4, S, D), dtype=np.float32)
    for b in range(4):
        partial = results[2 * b]["otp"].astype(np.float32) + results[
            2 * b + 1
        ]["otp"].astype(np.float32)  # [D, S]
        out[b] = partial.T
    return out


_NC_CACHE = None


def kernel(**inputs) -> np.ndarray:
    global _NC_CACHE
    from concourse.bass_utils import run_bass_kernel_spmd

    if _NC_CACHE is None:
        _NC_CACHE = build_kernel()
    in_maps = make_in_maps(
        inputs["query_input"],
        inputs["kv_input"],
        inputs["W_q"],
        inputs["W_k"],
        inputs["W_v"],
        inputs["W_o"],
    )
    res = run_bass_kernel_spmd(_NC_CACHE, in_maps, list(range(8)))
    return assemble_output(res.results)


# revision 34
# speedup vs baseline: 1.2003x; 1.0018x over previous
"""Multi-head attention (B=4, S=2048, D=1024, H=16, Dh=64) on 8 TRN2 cores.

Sharding: data-parallel over batch (4) x tensor-parallel over heads (2 groups
of 8). Core c handles batch c//2, head-group c%2 (heads g*8..g*8+7 via the
column split of W_q/W_k/W_v and row split of W_o). Each core emits a partial
output projection o^T [1024, 2048] bf16; the host sums the two head-group
partials per batch in f32 and transposes back.

All matmuls bf16 (inputs pre-cast on host), fp32 PSUM accumulation:
  Phase A (per head-pair ot): K^T, Q^T, V slices via bf16 matmuls. Chains run
  as PAIRS across psAO's two buffers, it-major interleaved, so each
  LDWEIGHTS hides under the partner chain's matmul stream.
  Phase B (per q-chunk qc, head-pair j):
    S^T tiles = K^T.T Q^T   (row-tiled head pairs at (0,0)/(64,0), K=64)
    P = exp(S/8)            (ACT per-ktile, PSUM[128,2,512] -> bf16)
    ctx^T += V.T P^T        (col-tiled pairs at (0,0)/(0,64), M=64)
    sums: DVE bf16 quarter-tree + col-tiled ones matmuls into one PSUM bank
    ctx^T *= 1/sums         (reciprocal_approx_fast + fused PSUM mul)
  Phase C: o^T = Wo_g^T.T @ ctx^T -> bf16 -> DMA out.

PSUM budget (8 banks): psS 2x2 + psC 1 + psR 1 + psAO 2 (projections and
output projection share a pool so phases overlap without false deps).
"""

import numpy as np

import concourse.bacc as bacc
import concourse.mybir as mybir
import concourse.tile as tile

D = 1024  # model dim
S = 2048  # sequence length
O = 512  # per-core projected dim (8 heads x 64)
IT = D // 128  # 8 input-dim tiles
NP = 4  # head pairs per core
QC = S // 512  # 4 q-chunks
KT = S // 128  # 16 k-tiles
VT = S // 128  # 16 s-tiles of V
MT = D // 128  # 8 output m-tiles
SCALE = 0.125  # 1/sqrt(64)

F32 = mybir.dt.float32
BF16 = mybir.dt.bfloat16


def build_kernel():
    nc = bacc.Bacc("TRN2", target_bir_lowering=False, debug=False, num_devices=8)
    xqt = nc.declare_dram_parameter("xqt", [QC, 128, IT, 512], BF16, isOutput=False)
    xkvt = nc.declare_dram_parameter("xkvt", [QC, 128, IT, 512], BF16, isOutput=False)
    wqt = nc.declare_dram_parameter("wqt", [128, IT, O], BF16, isOutput=False)
    wkt = nc.declare_dram_parameter("wkt", [128, IT, O], BF16, isOutput=False)
    wvt = nc.declare_dram_parameter("wvt", [128, IT, O], BF16, isOutput=False)
    wot = nc.declare_dram_parameter("wot", [128, O // 128, D], BF16, isOutput=False)
    ones = nc.declare_dram_parameter("ones", [128, 512], BF16, isOutput=False)
    otp = nc.declare_dram_parameter("otp", [D, S], BF16, isOutput=True)

    with tile.TileContext(nc) as tc:
        with (
            tc.tile_pool(name="persist", bufs=1) as persist,
            tc.tile_pool(name="ctxp", bufs=1) as ctxp,
            tc.tile_pool(name="pp", bufs=3) as ppp,
            tc.tile_pool(name="sump", bufs=2) as sump,
            tc.tile_pool(name="rp", bufs=2) as rp,
            tc.tile_pool(name="ots", bufs=3) as ots,
            tc.tile_pool(name="psS", bufs=2, space="PSUM") as psS,
            tc.tile_pool(name="psC", bufs=1, space="PSUM") as psC,
            tc.tile_pool(name="psR", bufs=1, space="PSUM") as psR,
            tc.tile_pool(name="psAO", bufs=2, space="PSUM") as psAO,
        ):
            qt_chunks = [
                [persist.tile([128, 512], BF16, name=f"qtc{j}_{c}") for c in range(QC)]
                for j in range(NP)
            ]
            kt_chunks = [
                [persist.tile([128, 512], BF16, name=f"ktc{j}_{c}") for c in range(QC)]
                for j in range(NP)
            ]
            v_tiles = [persist.tile([128, O], BF16, name=f"vt{t}") for t in range(VT)]
            wo_sb = persist.tile([128, O // 128, D], BF16)
            ones_full = persist.tile([128, 512], BF16)
            nc.scalar.dma_start(out=ones_full, in_=ones[:, :])
            ones_bf = ones_full[:, 0:64]

            # PE warm-up filler: lowest-priority dummy matmuls the scheduler
            # slots into PE-idle gaps (DMA head) to keep the HAM clock warm.
            warm_ps = psR.tile([128, 512], F32, tag="psR", name="warm_ps")
            with tc.high_priority(offset=-(10**6)):
                for _ in range(16):
                    nc.tensor.matmul(
                        warm_ps[0:64, :],
                        ones_bf,
                        ones_full,
                        start=True,
                        stop=True,
                    )

            # ---- Phase A: projections (overlaps phase B via disjoint pools) ----
            with tc.tile_pool(name="wqx", bufs=1) as wqp:
                wq_sb = wqp.tile([128, IT, O], BF16)
                wk_sb = wqp.tile([128, IT, O], BF16)
                wv_sb = wqp.tile([128, IT, O], BF16)
                # All input DMAs on the sync HW-DGE queue, in the order the
                # projection pipeline consumes them (K first, then Q, V, O).
                # First tiles split in halves so the first proj chain starts
                # before its full tile lands.
                xq_c, xkv_c = [], []
                # wk + wq ride the scalar engine's HW-DGE ring (idle until
                # the first exp), halving the 4MB critical head with the
                # activations streaming on the sync ring concurrently.
                nc.scalar.dma_start(out=wk_sb, in_=wkt[:, :, :])
                nc.scalar.dma_start(out=wq_sb, in_=wqt[:, :, :])
                xkv_1 = wqp.tile([128, IT, 512], BF16, name="xkv0")
                for h in range(4):
                    sl = slice(2 * h, 2 * h + 2)
                    nc.sync.dma_start(out=xkv_1[:, sl, :], in_=xkvt[0, :, sl, :])
                xkv_c.append(xkv_1)
                xq_1 = wqp.tile([128, IT, 512], BF16, name="xq0")
                for h in range(2):
                    sl = slice(4 * h, 4 * h + 4)
                    nc.sync.dma_start(out=xq_1[:, sl, :], in_=xqt[0, :, sl, :])
                xq_c.append(xq_1)
                for c in range(1, QC):
                    xkv_1 = wqp.tile([128, IT, 512], BF16, name=f"xkv{c}")
                    nc.sync.dma_start(out=xkv_1, in_=xkvt[c, :, :, :])
                    xkv_c.append(xkv_1)
                nc.sync.dma_start(out=wv_sb, in_=wvt[:, :, :])
                for c in range(1, QC):
                    xq_1 = wqp.tile([128, IT, 512], BF16, name=f"xq{c}")
                    nc.sync.dma_start(out=xq_1, in_=xqt[c, :, :, :])
                    xq_c.append(xq_1)
                nc.sync.dma_start(out=wo_sb, in_=wot[:, :, :])

                # ---- projection chain pairs: two psum chains (psAO bufs=2),
                # it-major interleaved so each LDW hides under the partner
                # chain's matmul stream. specs: (dst, 'kq'|'v', w/x args) ----
                _pn = [0]

                def proj_pair(specs):
                    pss = []
                    for _ in specs:
                        _pn[0] += 1
                        pss.append(
                            psAO.tile(
                                [128, 512], F32, tag="psAO", name=f"pj{_pn[0]}"
                            )
                        )
                    for it in range(IT):
                        for ps, spec in zip(pss, specs):
                            if spec[1] == "kq":
                                dst, _, w_sb, x_tile, ot = spec
                                nc.tensor.matmul(
                                    ps,
                                    w_sb[:, it, ot * 128 : (ot + 1) * 128],
                                    x_tile[:, it, :],
                                    start=(it == 0),
                                    stop=(it == IT - 1),
                                )
                            else:
                                dst, _, vt = spec
                                nc.tensor.matmul(
                                    ps,
                                    xkv_c[vt // 4][
                                        :, it, (vt % 4) * 128 : (vt % 4 + 1) * 128
                                    ],
                                    wv_sb[:, it, :],
                                    start=(it == 0),
                                    stop=(it == IT - 1),
                                )
                    for ps, spec in zip(pss, specs):
                        nc.vector.tensor_copy(spec[0], ps)

                def K(j, c):
                    return (kt_chunks[j][c], "kq", wk_sb, xkv_c[c], j)

                def Q(j, c):
                    return (qt_chunks[j][c], "kq", wq_sb, xq_c[c], j)

                def V(vt):
                    return (v_tiles[vt], "v", vt)

                # Head-pair 0's K/Q first so attention (and ACT) starts early;
                # V next (PV consumes v_tiles in kt order); rest of K/Q after.
                # Q chunks 2,3 are deferred into the attention phase (emitted
                # right before unit (2,0)) to offload the oversubscribed head.
                proj_pair([K(0, 0), Q(0, 0)])
                proj_pair([K(0, 1), K(0, 2)])
                proj_pair([K(0, 3), Q(0, 1)])
                for vt in range(0, VT // 2, 2):
                    proj_pair([V(vt), V(vt + 1)])
                proj_pair([K(1, 0), K(1, 1)])
                proj_pair([K(1, 2), K(1, 3)])
                proj_pair([Q(1, 0), Q(1, 1)])
                for vt in range(VT // 2, VT, 2):
                    proj_pair([V(vt), V(vt + 1)])
                # K/Q for head-pairs 2,3 are interleaved into the first
                # attention units below: their consumers (units (qc,2),(qc,3))
                # run 50us+ later, safely past the LDWEIGHTS pull-ahead
                # window, and this unloads the early-phase PE crunch.

                # ---- Phase B: attention (pair-outer); Phase C: output proj ----
                ctx_tiles = [
                    ctxp.tile([128, NP, 512], BF16, name=f"ctx{c}") for c in range(QC)
                ]
                # First qc pair interleaved j-major (buys the projection
                # pipeline runway); second half qc-major. out_proj(qc) is
                # emitted as soon as ctx(qc) is complete so its chains fill
                # the ACT-bound attention phase's PE slack, not the tail.
                def out_proj(qc, tail=False):
                    qsl = slice(qc * 512, (qc + 1) * 512)
                    for mt in range(MT):
                        ps_o = psAO.tile(
                            [128, 512], F32, tag="psAO", name=f"ps_o{qc}_{mt}"
                        )
                        for jt in range(NP):
                            nc.tensor.matmul(
                                ps_o,
                                wo_sb[:, jt, mt * 128 : (mt + 1) * 128],
                                ctx_tiles[qc][:, jt, :],
                                start=(jt == 0),
                                stop=(jt == NP - 1),
                            )
                        ot_sb = ots.tile([128, 512], BF16, tag="ot")
                        if tail and mt % 2 == 1:
                            # Tail copies alternate onto the (now idle)
                            # scalar engine so psAO slots free twice as fast.
                            nc.scalar.activation(
                                out=ot_sb,
                                in_=ps_o,
                                func=mybir.ActivationFunctionType.Copy,
                            )
                        else:
                            nc.vector.tensor_copy(ot_sb, ps_o)
                        nc.sync.dma_start(
                            out=otp[mt * 128 : (mt + 1) * 128, qsl], in_=ot_sb
                        )

                unit_order = [(qc, j) for j in range(NP) for qc in (0, 1)]
                unit_order += [(qc, j) for qc in (2, 3) for j in range(NP)]
                for ui, (qc, j) in enumerate(unit_order):
                    if (qc, j) == (1, 0):
                        proj_pair([K(2, 0), K(2, 1)])
                        proj_pair([K(2, 2), K(2, 3)])
                    if (qc, j) == (0, 1):
                        proj_pair([Q(2, 0), Q(2, 1)])
                        proj_pair([K(3, 0), K(3, 1)])
                    if (qc, j) == (1, 1):
                        proj_pair([K(3, 2), K(3, 3)])
                        proj_pair([Q(3, 0), Q(3, 1)])
                    if (qc, j) == (2, 0):
                        for ot in range(NP):
                            proj_pair([Q(ot, 2), Q(ot, 3)])
                    if (qc, j) == (2, 1):
                        out_proj(0)
                    if (qc, j) == (2, 3):
                        out_proj(1)
                    if (qc, j) == (3, 1):
                        out_proj(2)
                    if True:
                        # Alternate the ctx/sums banks per unit so unit n+1's
                        # PV chain doesn't wait on unit n's normalize read.
                        cp_, rp_ = (psC, psR) if ui % 2 == 0 else (psR, psC)
                        psum_ctx = cp_.tile(
                            [128, 512], F32, tag="psC" if ui % 2 == 0 else "psR"
                        )
                        psum_r = rp_.tile(
                            [128, 512], F32, tag="psR" if ui % 2 == 0 else "psC"
                        )
                        s1_tiles = []
                        sh_tiles = []
                        for qi in range(4):  # quarters of the k range
                            pq = ppp.tile([128, 4, 2, 512], BF16, tag="pp")
                            for ki in range(4):  # per-ktile ACT batches
                                kt = qi * 4 + ki
                                ps_s = psS.tile([128, 2, 512], F32, tag="psS")
                                with tc.high_priority(offset=50000):
                                    nc.tensor.matmul(
                                        ps_s[:, 0, :],
                                        kt_chunks[j][kt // 4][0:64, (kt % 4) * 128 : (kt % 4 + 1) * 128],
                                        qt_chunks[j][qc][0:64, :],
                                        start=True,
                                        stop=True,
                                        tile_position=(0, 0),
                                    )
                                    nc.tensor.matmul(
                                        ps_s[:, 1, :],
                                        kt_chunks[j][kt // 4][64:128, (kt % 4) * 128 : (kt % 4 + 1) * 128],
                                        qt_chunks[j][qc][64:128, :],
                                        start=True,
                                        stop=True,
                                        tile_position=(64, 0),
                                    )
                                    nc.scalar.activation(
                                        out=pq[:, ki, :, :],
                                        in_=ps_s[:, :, :],
                                        func=mybir.ActivationFunctionType.Exp,
                                        scale=SCALE,
                                    )
                            # PV + sums for this quarter
                            with tc.high_priority(offset=50000):
                                for ki in range(4):
                                    kt = qi * 4 + ki
                                    first = kt == 0
                                    last = kt == KT - 1
                                    nc.tensor.matmul(
                                        psum_ctx[0:64, :],
                                        v_tiles[kt][:, j * 128 : j * 128 + 64],
                                        pq[:, ki, 0, :],
                                        start=first,
                                        stop=last,
                                        tile_position=(0, 0),
                                    )
                                    nc.tensor.matmul(
                                        psum_ctx[64:128, :],
                                        v_tiles[kt][:, j * 128 + 64 : (j + 1) * 128],
                                        pq[:, ki, 1, :],
                                        start=first,
                                        stop=last,
                                        tile_position=(0, 64),
                                    )
                                tq = sump.tile([128, 2, 2, 512], BF16, tag="tq", bufs=1)
                                s1 = sump.tile([128, 2, 512], BF16, tag="s1")
                                with nc.allow_low_precision(reason="softmax sum partials"):
                                    nc.vector.tensor_add(
                                        tq, pq[:, 0:2, :, :], pq[:, 2:4, :, :]
                                    )
                                    nc.vector.tensor_add(
                                        s1, tq[:, 0, :, :], tq[:, 1, :, :]
                                    )
                                s1_tiles.append(s1)
                                if qi % 2 == 1:
                                    sh = sump.tile([128, 2, 512], BF16, tag="sh")
                                    with nc.allow_low_precision(reason="softmax sum partials"):
                                        nc.vector.tensor_add(
                                            sh, s1_tiles[qi - 1], s1_tiles[qi]
                                        )
                                    sh_tiles.append(sh)
                        s_all = sh_tiles[0]
                        with nc.allow_low_precision(reason="softmax sum partials"):
                            nc.vector.tensor_add(s_all, sh_tiles[0], sh_tiles[1])
                        with tc.high_priority(offset=50000):
                            nc.tensor.matmul(
                                psum_r[0:64, :],
                                ones_bf,
                                s_all[:, 0, :],
                                start=True,
                                stop=True,
                                tile_position=(0, 0),
                            )
                            nc.tensor.matmul(
                                psum_r[64:128, :],
                                ones_bf,
                                s_all[:, 1, :],
                                start=True,
                                stop=True,
                                tile_position=(0, 64),
                            )
                        # normalize
                        with tc.high_priority(offset=50000):
                            r_tile = rp.tile([128, 512], F32, tag="r")
                            nc.vector.reciprocal_approx_fast(out=r_tile, in_=psum_r)
                            with nc.allow_low_precision(reason="bf16 ctx for PE"):
                                nc.vector.tensor_mul(
                                    ctx_tiles[qc][:, j, :], psum_ctx, r_tile
                                )
                # Phase C tail: last q-chunk's output projection
                out_proj(3, tail=True)
    nc.compile()
    return nc


def make_in_maps(query_input, kv_input, W_q, W_k, W_v, W_o):
    import ml_dtypes

    bf16 = ml_dtypes.bfloat16
    q = np.asarray(query_input, dtype=np.float32).astype(bf16)
    kv = np.asarray(kv_input, dtype=np.float32).astype(bf16)
    W_q = np.asarray(W_q, dtype=np.float32).astype(bf16)
    W_k = np.asarray(W_k, dtype=np.float32).astype(bf16)
    W_v = np.asarray(W_v, dtype=np.float32).astype(bf16)
    W_o = np.asarray(W_o, dtype=np.float32).astype(bf16)
    ones = np.ones((128, 512), dtype=bf16)

    def tile_x(xt):  # [D, S] -> [QC, 128, IT, 512]
        return np.ascontiguousarray(
            xt.reshape(IT, 128, QC, 512).transpose(2, 1, 0, 3)
        )

    def tile_w(wt):  # [D, O] -> [128, IT, O]
        return np.ascontiguousarray(wt.reshape(IT, 128, O).transpose(1, 0, 2))

    in_maps = []
    for c in range(8):
        b, g = c // 2, c % 2
        sl = slice(g * O, (g + 1) * O)
        in_maps.append(
            {
                "xqt": tile_x(q[b].T),
                "xkvt": tile_x(kv[b].T),
                "wqt": tile_w(W_q[sl, :].T),
                "wkt": tile_w(W_k[sl, :].T),
                "wvt": tile_w(W_v[sl, :].T),
                "wot": np.ascontiguousarray(
                    W_o[:, sl].T.reshape(O // 128, 128, D).transpose(1, 0, 2)
                ),
                "ones": ones,
            }
        )
    return in_maps


def assemble_output(results):
    out = np.empty((4, S, D), dtype=np.float32)
    for b in range(4):
        partial = results[2 * b]["otp"].astype(np.float32) + results[
            2 * b + 1
        ]["otp"].astype(np.float32)  # [D, S]
        out[b] = partial.T
    return out


_NC_CACHE = None


def kernel(**inputs) -> np.ndarray:
    global _NC_CACHE
    from concourse.bass_utils import run_bass_kernel_spmd

    if _NC_CACHE is None:
        _NC_CACHE = build_kernel()
    in_maps = make_in_maps(
        inputs["query_input"],
        inputs["kv_input"],
        inputs["W_q"],
        inputs["W_k"],
        inputs["W_v"],
        inputs["W_o"],
    )
    res = run_bass_kernel_spmd(_NC_CACHE, in_maps, list(range(8)))
    return assemble_output(res.results)
